# revision 40
# baseline (speedup 1.0000x reference)
"""Trainium2 Bass kernel for nn_Meta_67078799229377 (relation-network meta-learner).

Sharding: 8 cores = 4 batch elements x 2 halves of the relation-j axis.
v2: packed input DMAs + PE warmup, row/col-tiled convs, balanced DVE/ACT
relation stage (hdd on DVE, fused relu+sum z-blocks on ACT), matmul tail.
"""
import os
import numpy as np
import ml_dtypes

import concourse.bass as bass
import concourse.mybir as mybir
import concourse.tile as tile
from concourse import bacc
from concourse.bass_utils import run_bass_kernel_spmd

F32 = mybir.dt.float32
BF16 = mybir.dt.bfloat16
AF = mybir.ActivationFunctionType
OP = mybir.AluOpType

B, S, D = 4, 6, 8
M = D * D
C2 = 66
H1 = 128
CO = 64
NCls = 64
N_CORES = 8
_W = None

# wb (bf16 pack) column offsets
W1C, W2C, W3C = 0, 32, 464
W1A, W1B, WG2, WF1, WF2 = 1040, 1168, 1296, 1360, 1376
CRD = 1377
NB = 1761
# wf (f32 pack) column offsets
BC1, BC2, BC3, BG1, BG2, BF1, NBF2, WLE, OH = 0, 1, 2, 3, 4, 5, 6, 7, 71
NF = 135

WARMUP_MMS = int(os.environ.get("KWARM", "9"))
# per-unit hdd q-ops routed to the scalar engine: front-load unit 0 (ACT idle
# until the first z-block), tiny share mid-units, extra in the last unit
ACT_QS_BY_UNIT = {
    0: (9, 11, 17, 19, 25, 27, 29, 31),
    1: (19, 27), 2: (19, 27), 3: (19, 27), 4: (19, 27),
}
# last unit: middle duo's relu+sum runs on DVE to parallelize the drain
DVE_Z = set()


def _build_nc():
    nc = bacc.Bacc("TRN2", target_bir_lowering=False, debug=False,
                   num_devices=N_CORES)

    x_wb = nc.dram_tensor("wb", [128, NB], BF16, kind="ExternalInput")
    x_wf = nc.dram_tensor("wf", [128, NF], F32, kind="ExternalInput")
    x_pt = nc.dram_tensor("pt", [128, 2, 2, 512], BF16, kind="ExternalInput")
    out_scores = nc.dram_tensor("xf", [128, 18], F32, kind="ExternalOutput")
    out_cls = nc.dram_tensor("logits", [S, NCls], F32, kind="ExternalOutput")

    with tile.TileContext(nc) as tc:
        with (
            tc.tile_pool(name="const", bufs=1) as cpool,
            tc.tile_pool(name="work", bufs=1) as wpool,
            tc.tile_pool(name="hdd", bufs=3) as hpool,
            tc.tile_pool(name="gscr", bufs=2) as spool,
            tc.tile_pool(name="pz", bufs=2, space="PSUM") as pz,
        ):
            # ---- inputs (posted first so transfers start ASAP) ----
            wb = cpool.tile([128, NB], BF16)
            nc.sync.dma_start(out=wb[:], in_=x_wb[:])
            pt = cpool.tile([128, 2, 2, 512], BF16)
            nc.gpsimd.dma_start(out=pt[:], in_=x_pt[:])
            wf = cpool.tile([128, NF], F32)
            nc.scalar.dma_start(out=wf[:], in_=x_wf[:])

            # ---- PE warmup + ACT table preload (no input deps) ----
            warm = wpool.tile([128, 576], BF16)
            nc.gpsimd.memset(warm[:], 0.001)
            dum = wpool.tile([2, 4], F32)
            nc.scalar.activation(dum[:, 0:2], warm[0:2, 0:2], AF.Relu)
            pswarm = pz.tile([128, 2048], F32, tag="z")
            for r in range(WARMUP_MMS):
                nc.tensor.matmul(pswarm[0:64, 0:512], warm[:, 0:64],
                                 warm[:, 64:576], start=True, stop=True)

            # ---- conv buffers + padding ----
            c1a = wpool.tile([128, 33, 33], BF16)   # imgs 0-3, ch at part 32k
            c1b = wpool.tile([64, 33, 33], BF16)    # imgs 4-5
            nc.gpsimd.memset(c1a[:, 32, :], 0.0)
            nc.gpsimd.memset(c1a[:, 0:32, 32], 0.0)
            nc.gpsimd.memset(c1b[:, 32, :], 0.0)
            nc.gpsimd.memset(c1b[:, 0:32, 32], 0.0)
            c2f = wpool.tile([128, 4, 17, 17], BF16)  # imgs0-3 @0:48, 4-5 @64:112
            nc.gpsimd.memset(c2f[:, :, 16, :], 0.0)
            nc.gpsimd.memset(c2f[:, :, 0:16, 16], 0.0)

            # ---- conv1: 27->32ch, 64x64 -> 32x32, diag-tiled 4 imgs ----
            c1ps = pz.tile([128, 2, 16, 32], F32, tag="z")
            for h in range(2):
                for k in range(4):
                    nc.tensor.matmul(
                        c1ps[32 * k:32 * k + 32, h, :, :],
                        wb[32 * k:32 * k + 27, W1C:W1C + 32],
                        pt[32 * k:32 * k + 27, h, 0, :],
                        start=True, stop=True, tile_position=(32 * k, 32 * k))
            nc.scalar.activation(
                c1a[:, 0:32, 0:32].rearrange("p (h y) x -> p h y x", h=2),
                c1ps[:], AF.Relu, bias=wf[:, BC1:BC1 + 1])
            c1psb = pz.tile([64, 2, 16, 32], F32, tag="z")
            for h in range(2):
                for k in range(2):
                    nc.tensor.matmul(
                        c1psb[32 * k:32 * k + 32, h, :, :],
                        wb[32 * k:32 * k + 27, W1C:W1C + 32],
                        pt[32 * k:32 * k + 27, h, 1, :],
                        start=True, stop=True)
            nc.vector.tensor_scalar(
                c1b[:, 0:32, 0:32].rearrange("p (h y) x -> p h y x", h=2),
                c1psb[:], wf[0:64, BC1:BC1 + 1], 0.0, op0=OP.add, op1=OP.max)

            # ---- conv2: 32->48ch, 32x32 -> 16x16, 4-way row-tiled ----
            c2ps = pz.tile([48, 4, 512], F32, tag="z")
            taps = [(dy, dx) for dy in range(3) for dx in range(3)]
            for t, (dy, dx) in enumerate(taps):
                for k in range(4):
                    nc.tensor.matmul(
                        c2ps[:, k, 0:256].rearrange("p (y x) -> p y x", y=16),
                        wb[32 * k:32 * k + 32, W2C + 48 * t:W2C + 48 * (t + 1)],
                        c1a[32 * k:32 * k + 32, dy:dy + 31:2, dx:dx + 31:2],
                        start=(t == 0), stop=(t == 8), skip_group_check=True,
                        tile_position=(32 * k, 0))
            nc.scalar.activation(
                c2f[0:48, :, 0:16, 0:16],
                c2ps[:, :, 0:256].rearrange("p k (y x) -> p k y x", y=16),
                AF.Relu, bias=wf[0:48, BC2:BC2 + 1])
            c2psb = pz.tile([128, 2, 512], F32, tag="z")
            for t, (dy, dx) in enumerate(taps):
                for k in range(2):
                    nc.tensor.matmul(
                        c2psb[64:112, k, 0:256].rearrange("p (y x) -> p y x", y=16),
                        wb[32 * k:32 * k + 32, W2C + 48 * t:W2C + 48 * (t + 1)],
                        c1b[32 * k:32 * k + 32, dy:dy + 31:2, dx:dx + 31:2],
                        start=(t == 0), stop=(t == 8), skip_group_check=True)
            nc.vector.tensor_scalar(
                c2f[64:112, 0:2, 0:16, 0:16],
                c2psb[64:112, :, 0:256].rearrange("p k (y x) -> p k y x", y=16),
                wf[64:112, BC2:BC2 + 1], 0.0, op0=OP.add, op1=OP.max)

            # ---- conv3: 48->64ch, 16x16 -> 8x8, 2 row-tile streams ----
            c3psa = pz.tile([64, 4, 64], F32, tag="z")
            c3psb = pz.tile([64, 2, 64], F32, tag="z")
            for t, (dy, dx) in enumerate(taps):
                nc.tensor.matmul(
                    c3psa[:].rearrange("p k (y x) -> p k y x", y=8),
                    wb[0:48, W3C + 64 * t:W3C + 64 * (t + 1)],
                    c2f[0:48, :, dy:dy + 15:2, dx:dx + 15:2],
                    start=(t == 0), stop=(t == 8), skip_group_check=True)
                nc.tensor.matmul(
                    c3psb[:].rearrange("p k (y x) -> p k y x", y=8),
                    wb[64:112, W3C + 64 * t:W3C + 64 * (t + 1)],
                    c2f[64:112, 0:2, dy:dy + 15:2, dx:dx + 15:2],
                    start=(t == 0), stop=(t == 8), skip_group_check=True)
            featc = wpool.tile([66, 6, M], BF16)
            nc.scalar.activation(featc[0:64, 0:4, :], c3psa[:], AF.Relu,
                                 bias=wf[0:64, BC3:BC3 + 1])
            nc.vector.tensor_scalar(featc[0:64, 4:6, :], c3psb[:],
                                    wf[0:64, BC3:BC3 + 1], 0.0,
                                    op0=OP.add, op1=OP.max)
            nc.vector.tensor_copy(featc[64:66, :, :],
                                  wb[64:66, CRD:CRD + 384]
                                  .rearrange("p (i m) -> p i m", m=M))

            # ---- u / v ----
            fc = featc[:].rearrange("p i m -> p (i m)")
            psu = pz.tile([128, 384], F32, tag="z")
            nc.tensor.matmul(psu[:], wb[0:66, W1A:W1A + 128], fc,
                             start=True, stop=True)
            psv = pz.tile([128, 384], F32, tag="z")
            nc.tensor.matmul(psv[:], wb[0:66, W1B:W1B + 128], fc,
                             start=True, stop=True)
            u_sb = wpool.tile([128, 384], F32)
            nc.scalar.activation(u_sb[:], psu[:], AF.Copy)
            v_bf = wpool.tile([128, 384], BF16)
            nc.vector.tensor_scalar(v_bf[:], psv[:], wf[:, BG1:BG1 + 1], None,
                                    op0=OP.add)

            # ---- cls head (runs whenever engines are free) ----
            fme = wpool.tile([65, S], F32)
            nc.gpsimd.memset(fme[64:65, :], 1.0)
            nc.vector.tensor_reduce(fme[0:64, :], featc[0:64, :, :],
                                    axis=mybir.AxisListType.X, op=OP.add)
            psl = pz.tile([S, NCls], F32, tag="z")
            nc.tensor.matmul(psl[:], fme[:], wf[0:65, WLE:WLE + 64],
                             start=True, stop=True)
            lsb = wpool.tile([S, NCls], F32)
            nc.vector.tensor_copy(lsb[:], psl[:])
            nc.sync.dma_start(out=out_cls[:], in_=lsb[:])
            psf0 = pz.tile([128, 2048], F32, tag="z")
            for r in range(4):
                nc.tensor.matmul(psf0[0:64, 0:512], warm[:, 0:64],
                                 warm[:, 64:576], start=True, stop=True)

            # ---- relation units ----
            xf_cols = wpool.tile([128, 18], F32)
            for jl in range(3):
                for qh in range(2):
                    unit = jl * 2 + qh
                    act_qs = ACT_QS_BY_UNIT.get(unit, ())
                    hdd = hpool.tile([128, 32, 384], BF16, tag="hdd")
                    for ql in range(32):
                        q = qh * 32 + ql
                        ucol = u_sb[:, jl * M + q:jl * M + q + 1]
                        if ql in act_qs:
                            nc.scalar.activation(hdd[:, ql, :], v_bf[:],
                                                 AF.Relu, bias=ucol)
                        else:
                            nc.vector.tensor_scalar(hdd[:, ql, :], v_bf[:],
                                                    ucol, 0.0,
                                                    op0=OP.add, op1=OP.max)
                    for duo in range(3):
                        iA, iB = 2 * duo, 2 * duo + 1
                        zps = pz.tile([128, 2048], F32, tag="z")
                        for ch in range(4):
                            q0 = ch * 8
                            nc.tensor.matmul(
                                zps[0:CO, 512 * ch:512 * (ch + 1)],
                                wb[:, WG2:WG2 + 64],
                                hdd[:, q0:q0 + 8, iA * M:(iA + 1) * M],
                                start=True, stop=True)
                            nc.tensor.matmul(
                                zps[CO:2 * CO, 512 * ch:512 * (ch + 1)],
                                wb[:, WG2:WG2 + 64],
                                hdd[:, q0:q0 + 8, iB * M:(iB + 1) * M],
                                start=True, stop=True)
                        gscr = spool.tile([128, 2048], BF16, tag="gscr")
                        col = unit * 3 + duo
                        nc.scalar.activation(
                            gscr[:], zps[:], AF.Relu,
                            bias=wf[:, BG2:BG2 + 1],
                            accum_out=xf_cols[:, col:col + 1])
                    if unit < 5:
                        psfu = pz.tile([128, 2048], F32, tag="z",
                                       name=f"psfu{unit}")
                        for r in range(3):
                            nc.tensor.matmul(psfu[0:64, 0:512], warm[:, 0:64],
                                             warm[:, 64:576],
                                             start=True, stop=True)

            # ---- score head on host: ship raw xf accumulators ----
            nc.sync.dma_start(out=out_scores[:], in_=xf_cols[:])
    nc.compile()
    return nc


_NC_CACHE = None


def _get_nc():
    global _NC_CACHE
    if _NC_CACHE is None:
        _NC_CACHE = _build_nc()
    return _NC_CACHE


def _host_prep(inputs):
    f32 = np.float32
    bf16 = ml_dtypes.bfloat16
    ins = {k: np.asarray(v) for k, v in inputs.items()}
    x = np.concatenate([ins['support_x'], ins['query_x']], axis=1)
    lab = np.concatenate([ins['support_y'], ins['query_y']], axis=1)

    xpad = np.pad(x.astype(f32), ((0, 0), (0, 0), (0, 0), (0, 1), (0, 1)))
    win = np.lib.stride_tricks.sliding_window_view(xpad, (3, 3), axis=(3, 4))
    win = win[:, :, :, ::2, ::2]
    # [B, 27, S, 1024]
    patches = win.transpose(0, 2, 5, 6, 1, 3, 4).reshape(B, 27, S, 1024)
    patches = np.ascontiguousarray(patches, f32)

    w1 = np.ascontiguousarray(ins['k1'].reshape(32, 27).T, f32)
    w2 = np.ascontiguousarray(
        ins['k2'].transpose(1, 2, 3, 0).reshape(32, 9 * 48), f32)
    w3 = np.ascontiguousarray(
        ins['k3'].transpose(1, 2, 3, 0).reshape(48, 9 * 64), f32)
    Wg1 = ins['Wg1'].astype(f32)

    wb = np.zeros((128, NB), f32)
    for k in range(4):
        wb[32 * k:32 * k + 27, W1C:W1C + 32] = w1
        wb[32 * k:32 * k + 32, W2C:W2C + 432] = w2
    wb[0:48, W3C:W3C + 576] = w3
    wb[64:112, W3C:W3C + 576] = w3
    wb[0:66, W1A:W1A + 128] = Wg1[:C2]
    wb[0:66, W1B:W1B + 128] = Wg1[C2:]
    wb[0:128, WG2:WG2 + 64] = ins['Wg2'].astype(f32)
    wb[0:64, WF1:WF1 + 16] = ins['Wf1'].astype(f32)
    wb[64:128, WF1:WF1 + 16] = ins['Wf1'].astype(f32)
    wb[0:16, WF2:WF2 + 1] = ins['Wf2'].astype(f32)
    ii = np.arange(D, dtype=f32) / D
    coord = np.stack([np.broadcast_to(ii[:, None], (D, D)),
                      np.broadcast_to(ii[None, :], (D, D))]).reshape(2, M)
    wb[64:66, CRD:CRD + 384] = np.tile(coord, (1, S))
    wb = wb.astype(bf16)

    wfc = np.zeros((128, NF), f32)
    wfc[0:32, BC1] = ins['bc1'].astype(f32)
    wfc[0:48, BC2] = ins['bc2'].astype(f32)
    wfc[0:64, BC3] = ins['bc3'].astype(f32)
    wfc[64:128, BC3] = ins['bc3'].astype(f32)
    wfc[:, BG1] = ins['bg1'].astype(f32)
    wfc[0:64, BG2] = ins['bg2'].astype(f32)
    wfc[64:128, BG2] = ins['bg2'].astype(f32)
    wfc[0:16, BF1] = ins['bf1'].astype(f32)
    wfc[:, NBF2] = -float(ins['bf2'][0])
    wfc[0:64, WLE:WLE + 64] = ins['Wlog'].astype(f32) / M
    wfc[64, WLE:WLE + 64] = ins['blog'].astype(f32)

    onehots = np.zeros((B, S, NCls), f32)
    for b in range(B):
        onehots[b, np.arange(S), lab[b]] = 1.0

    global _W
    _W = {'Wf1': ins['Wf1'].astype(np.float64),
          'bf1': ins['bf1'].astype(np.float64),
          'Wf2': ins['Wf2'].astype(np.float64)}
    in_maps = []
    for core in range(N_CORES):
        b, half = core // 2, core % 2
        perm = (0, 1, 2, 3, 4, 5) if half == 0 else (3, 4, 5, 0, 1, 2)
        p = patches[b][:, perm, :]          # [27, 6, 1024]
        pc = np.zeros((128, 2, 2, 512), f32)
        for k in range(4):
            pc[32 * k:32 * k + 27, :, 0, :] = \
                p[:, k, :].reshape(27, 2, 512)
        pc[0:27, :, 1, :] = p[:, 4, :].reshape(27, 2, 512)
        pc[32:59, :, 1, :] = p[:, 5, :].reshape(27, 2, 512)
        wfi = wfc.copy()
        wfi[0:S, OH:OH + 64] = onehots[b][list(perm)]
        in_maps.append(dict(wb=wb, wf=wfi, pt=pc.astype(bf16)))
    return in_maps, lab, ins['bf2'].astype(f32)


def _host_post(results, lab, bf2):
    P = np.zeros((B, S, S), np.float32)
    cls_terms = np.zeros((B, S), np.float32)
    for core in range(N_CORES):
        b, half = core // 2, core % 2
        perm = (0, 1, 2, 3, 4, 5) if half == 0 else (3, 4, 5, 0, 1, 2)
        xf = results[core]["xf"].astype(np.float64)      # [128, 18]
        for jl in range(3):
            for duo in range(3):
                for ihalf in range(2):
                    rows = slice(64 * ihalf, 64 * ihalf + 64)
                    xp = (xf[rows, (jl * 2) * 3 + duo]
                          + xf[rows, (jl * 2 + 1) * 3 + duo])
                    h1 = np.maximum(xp @ _W['Wf1'] + _W['bf1'], 0.0)
                    raw = float(h1 @ _W['Wf2'][:, 0]) + float(bf2[0])
                    sc = 1.0 / (1.0 + np.exp(-raw))
                    P[b, perm[2 * duo + ihalf], perm[jl]] = np.float32(sc)
        if half == 0:
            lg = results[core]["logits"].reshape(S, NCls).astype(np.float64)
            mx = lg.max(axis=1, keepdims=True)
            lse = np.log(np.exp(lg - mx).sum(axis=1)) + mx[:, 0]
            cls_terms[b] = (lse - lg[np.arange(S), lab[b]]).astype(np.float32)
    cls_loss = np.float32(cls_terms.mean())
    y = (lab[:, :, None] == lab[:, None, :]).astype(np.float32)
    Pt = P.transpose(0, 2, 1)
    sym, anti = np.float32(0.5) * (P + Pt), np.float32(0.5) * (P - Pt)
    sym_n = np.sqrt((sym ** 2).sum(axis=(1, 2)))
    anti_n = np.sqrt((anti ** 2).sum(axis=(1, 2)))
    sym_loss = np.float32(((sym_n - anti_n) / (sym_n + anti_n)).mean())
    euc_loss = np.float32(((P - y) ** 2).mean())
    rn_loss = np.float32(euc_loss - np.float32(0.1) * sym_loss)
    return np.float32(cls_loss), np.float32(rn_loss), np.float32(sym_loss)


def run_spmd(inputs, trace=False, **kwargs):
    nc = _get_nc()
    in_maps, lab, bf2 = _host_prep(inputs)
    res = run_bass_kernel_spmd(nc, in_maps, list(range(N_CORES)),
                               trace=trace, **kwargs)
    return _host_post(res.results, lab, bf2), res


def kernel(**inputs):
    out, _ = run_spmd(inputs)
    return out
